# revision 6
# baseline (speedup 1.0000x reference)
"""Trainium2 Bass kernel for MessagePassingWithPhase (v3).

Reference computation (B=2, N=512, D=128, O=4):
    recv = X @ W1r ; send = X @ W1s
    hidden[b,i,j,:]  = relu(recv[b,i] + send[b,j] + b1)
    messages         = hidden @ W2 + b2
    gate             = sigmoid(cos(phi_i - phi_j) @ Wg + bg)
    agg[b,i]         = sum_j mask[i,j] * (messages * gate)[b,i,j] / cnt_i
    out              = X + (relu(X@Wu1x + agg@Wu1a + bu1) @ Wu2 + bu2)

Mapping: 8 cores, receiver axis sharded (64 receivers/core, both batches),
feature-major (D=128 partitions, node index on the free axis).

v3 design (vs the v2 baseline's serialized PE->ACT->GPSIMD->DVE chain):
  * H = relu(send_j + recv_i + b1) is built directly from SBUF by the
    ACT/DVE/GPSIMD engines using the per-partition bias/scalar operand
    (bias = recv_i + b1 column) -- the v2 identity-matmul H-build (40% of
    all PE columns) is gone.
  * The gate linear, the adjacency mask (-48 saturation), and nothing else
    run as ONE fp16 K=32 matmul per receiver, row-tiled via tile_position
    so 4 receivers' gate matmuls execute concurrently in disjoint 32-row
    strips of the PE array.  The mask row rides along as contraction row
    32g+8 (stationary=1, rhs=-48*(1-mask)).
  * messages-evac + b2 + gate-multiply + the whole sum_j reduction collapse
    into ONE DVE scalar_tensor_tensor per receiver:
        out = (mq_psum + b2) * Gsb ; accum_out = sum_j -> araw column.
  * PSUM is double-buffered ((D,2N) gq/mq tags x2 bufs = 8 banks) so PE
    never waits on evacuation; relu H-builds are distributed across
    ACT/DVE/GPSIMD by RELU_POLICY to balance engine load.
"""
import os
import sys
import numpy as np

for _p in ("/opt/trn_rl_repo", "/root/.axon_site/_ro/trn_rl_repo"):
    if os.path.isdir(_p) and _p not in sys.path:
        sys.path.append(_p)

B, N, D, O = 2, 512, 128, 4
NCORES = 8
NPC = N // NCORES       # receivers per core (per batch)
NCH = NPC // 4          # 4-receiver groups per batch (p8m/wi chunking)
NC2 = NPC // 2          # 2-receiver PSUM chunks per batch
MASK_NEG = -48.0        # sigmoid(-48+eps) ~ 1e-20 -> masked-mean == sum

REPEAT = 1              # timing aid: loop trip count of the device For_i

# per-receiver engine for the relu H-build: s=ACT, v=DVE, g=GPSIMD.
# All H-builds run as a dependency-free phase up front (every engine's
# FIFO can drain them without cross-engine stalls), so the policy only
# balances total load.
RELU_POLICY = "gvsv"

# timing-ablation: stages to OMIT (timing experiments only; output wrong)
ABLATE = frozenset()

_CACHE = {}


def _build_program():
    import concourse.bacc as bacc
    import concourse.mybir as mybir
    import concourse.tile as tile

    f32 = mybir.dt.float32
    f16 = mybir.dt.float16
    A = mybir.AluOpType
    AF = mybir.ActivationFunctionType

    nc = bacc.Bacc("TRN2", debug=False)

    def din(name, shape, dt=f32):
        return nc.declare_dram_parameter(name, list(shape), dt, isOutput=False)

    xt = din("xt", (B, D, N))            # node features, transposed
    xtr = din("xtr", (D, B * NPC))       # receiver cols of xt, both batches
    p8m = din("p8m", (B, D, NCH * N), f16)   # gate rhs: P8 rows + mask row
    p8rch = din("p8rch", (B, D, NCH), f16)   # receiver phases packed + ones
    wg8sh = din("wg8sh", (D, D), f16)        # Wg stacked per group + ones
    w2h = din("w2h", (D, D), f16)
    # [w1r|w1s|wu1x|wu1a|wu2|cinv(B*NPC)|b1|bg|b2|bu1|bu2]
    NBLOB = 5 * D + B * NPC + 5
    blob = din("blob", (D, NBLOB))
    out = nc.declare_dram_parameter("out", [B, D, NPC], f32, isOutput=True)

    ab = ABLATE

    with tile.TileContext(nc) as tc:
        with (
            tc.tile_pool(name="const", bufs=1) as cp,
            tc.tile_pool(name="work", bufs=1) as wp,
            tc.tile_pool(name="psA", bufs=2, space="PSUM") as psA,
        ):
            def ct(dram, shape, dt=f32, tag=None):
                t = cp.tile(list(shape), dt, tag=tag, name=tag)
                nc.sync.dma_start(t[:], dram[:])
                return t

            blob_t = ct(blob, (D, NBLOB), tag="blob")
            w1r_t = blob_t[:, 0 * D:1 * D]
            w1s_t = blob_t[:, 1 * D:2 * D]
            wu1x_t = blob_t[:, 2 * D:3 * D]
            wu1a_t = blob_t[:, 3 * D:4 * D]
            wu2_t = blob_t[:, 4 * D:5 * D]
            cinv_t = blob_t[:, 5 * D:5 * D + B * NPC]
            bofs = 5 * D + B * NPC
            b1c = blob_t[:, bofs + 0:bofs + 1]
            bgc = blob_t[:, bofs + 1:bofs + 2]
            b2c = blob_t[:, bofs + 2:bofs + 3]
            bu1c = blob_t[:, bofs + 3:bofs + 4]
            bu2c = blob_t[:, bofs + 4:bofs + 5]

            w2h_t = ct(w2h, (D, D), f16, tag="w2h")
            wg8s_t = ct(wg8sh, (D, D), f16, tag="wg8sh")
            xt_t, p8m_t, p8rc_t = [], [], []
            for b in range(B):
                xt_t.append(ct(xt[b], (D, N), tag=f"xt{b}"))
                p8m_t.append(ct(p8m[b], (D, NCH * N), f16, tag=f"p8m{b}"))
                p8rc_t.append(ct(p8rch[b], (D, NCH), f16, tag=f"p8rc{b}"))
            xtr_all = ct(xtr, (D, B * NPC), tag="xtr")

            sendT = [cp.tile([D, N], f32, tag=f"send{b}", name=f"send{b}")
                     for b in range(B)]
            recvb1 = cp.tile([D, B * NPC], f32, tag="recvb1", name="recvb1")
            wi_pack = [cp.tile([D, NCH * D], f16, tag=f"wi{b}", name=f"wi{b}")
                       for b in range(B)]
            araw = cp.tile([D, B * NPC], f32, tag="araw", name="araw")
            o_all = cp.tile([D, B * NPC], f32, tag="o_all", name="o_all")

            relu_eng = {"s": nc.scalar, "v": nc.vector, "g": nc.gpsimd}

            chunks = [(b, c2) for b in range(B) for c2 in range(NC2)]
            NCHK = len(chunks)

            with tc.For_i(0, REPEAT, 1):
                # ---- projections ----
                for b in range(B if "proj" not in ab else 0):
                    s_ps = psA.tile([D, N], f32, tag="mq", name="s_ps")
                    nc.tensor.matmul(s_ps[:], w1s_t, xt_t[b][:],
                                     start=True, stop=True)
                    nc.scalar.copy(sendT[b][:], s_ps[:])
                if "proj" not in ab:
                    r_ps = psA.tile([D, B * NPC], f32, tag="gq", name="r_ps")
                    nc.tensor.matmul(r_ps[:], w1r_t, xtr_all[:],
                                     start=True, stop=True)
                    nc.scalar.add(recvb1[:], r_ps[:], b1c)
                else:
                    nc.vector.tensor_copy(recvb1[:], xtr_all[:])
                    for b in range(B):
                        nc.scalar.copy(sendT[b][:], xt_t[b][:])

                # ---- per-receiver gate stationaries (+ ones mask row) ----
                for b in range(B if "wi" not in ab else 0):
                    nc.vector.tensor_tensor(
                        wi_pack[b][:].rearrange("p (c d) -> p c d", c=NCH),
                        wg8s_t.unsqueeze(1).broadcast_to((D, NCH, D)),
                        p8rc_t[b][:].unsqueeze(2).broadcast_to((D, NCH, D)),
                        A.mult)
                if "wi" in ab:
                    for b in range(B):
                        nc.vector.tensor_copy(
                            wi_pack[b][:, 0:D],
                            wg8s_t)

                # ---- relu H-build phase: all 128 receivers, no deps ----
                Ht = {}
                for ci in range(NCHK):
                    bb, cc2 = chunks[ci]
                    Ht[ci] = wp.tile([D, 2 * N], f16, tag="H", bufs=NCHK,
                                     name="H")
                    if "relu" in ab:
                        nc.vector.memset(Ht[ci][:], 0.0)
                        continue
                    for k in range(2):
                        r = 2 * cc2 + k
                        col = bb * NPC + r
                        eng = relu_eng[RELU_POLICY[(ci * 2 + k)
                                                   % len(RELU_POLICY)]]
                        dst = Ht[ci][:, k * N:(k + 1) * N]
                        if eng is nc.scalar:
                            nc.scalar.activation(
                                dst, sendT[bb][:], AF.Relu,
                                bias=recvb1[:, col:col + 1])
                        else:
                            eng.tensor_scalar(
                                dst, sendT[bb][:],
                                recvb1[:, col:col + 1], 0.0,
                                A.add, A.max)

                # ---- chunk pipeline ----
                for ci in range(NCHK):
                    b, c2 = chunks[ci]

                    gq = psA.tile([D, 2 * N], f32, tag="gq", name="gq")
                    for k in range(2 if "gate" not in ab else 1):
                        r = 2 * c2 + k
                        g, c16 = r % 4, r // 4
                        nc.tensor.matmul(
                            gq[:, k * N:(k + 1) * N],
                            wi_pack[b][32 * g:32 * g + 32,
                                       c16 * D:(c16 + 1) * D],
                            p8m_t[b][32 * g:32 * g + 32,
                                     c16 * N:(c16 + 1) * N],
                            start=True, stop=True,
                            tile_position=(32 * g, 0))
                    if "gate" in ab:
                        nc.tensor.matmul(
                            gq[:, N:2 * N],
                            wi_pack[b][0:32, 0:D],
                            p8m_t[b][0:32, 0:N],
                            start=True, stop=True, tile_position=(0, 0))

                    Gsb = wp.tile([D, 2 * N], f16, tag="Gsb", bufs=2,
                                  name="Gsb")
                    if "sig" not in ab:
                        nc.scalar.activation(Gsb[:], gq[:], AF.Sigmoid,
                                             bias=bgc)
                    else:
                        nc.vector.memset(Gsb[:], 1.0)

                    mq = psA.tile([D, 2 * N], f32, tag="mq", name="mq")
                    for k in range(2 if "msg" not in ab else 1):
                        nc.tensor.matmul(
                            mq[:, k * N:(k + 1) * N], w2h_t[:],
                            Ht[ci][:, k * N:(k + 1) * N],
                            start=True, stop=True)
                    if "msg" in ab:
                        nc.tensor.matmul(
                            mq[:, N:2 * N], w2h_t[:], Ht[ci][:, 0:N],
                            start=True, stop=True)

                    scr = wp.tile([D, 2 * N], f16, tag="scr", bufs=2,
                                  name="scr")
                    for k in range(2 if "stt" not in ab else 0):
                        r = 2 * c2 + k
                        col = b * NPC + r
                        nc.vector.scalar_tensor_tensor(
                            scr[:, k * N:(k + 1) * N],
                            mq[:, k * N:(k + 1) * N],
                            b2c,
                            Gsb[:, k * N:(k + 1) * N],
                            A.add, A.mult,
                            accum_out=araw[:, col:col + 1])
                    del Ht[ci]
                if "stt" in ab:
                    nc.vector.memset(araw[:], 1.0)

                # ---- masked-mean scale + update MLP + residual ----
                if "tail" in ab:
                    nc.vector.tensor_copy(o_all[:], xtr_all[:])
                else:
                    aggt = wp.tile([D, B * NPC], f32, tag="aggt", name="aggt")
                    nc.vector.tensor_tensor(aggt[:], araw[:], cinv_t, A.mult)
                    u_ps = psA.tile([D, B * NPC], f32, tag="mq", name="u_ps")
                    nc.tensor.matmul(u_ps[:], wu1x_t, xtr_all[:],
                                     start=True, stop=False)
                    nc.tensor.matmul(u_ps[:], wu1a_t, aggt[:],
                                     start=False, stop=True)
                    hT = wp.tile([D, B * NPC], f32, tag="hT", name="hT")
                    nc.scalar.activation(hT[:], u_ps[:], AF.Relu, bias=bu1c)
                    o_ps = psA.tile([D, B * NPC], f32, tag="gq", name="o_ps")
                    nc.tensor.matmul(o_ps[:], wu2_t, hT[:],
                                     start=True, stop=True)
                    o1 = wp.tile([D, B * NPC], f32, tag="o1", name="o1")
                    nc.scalar.add(o1[:], o_ps[:], bu2c)
                    nc.vector.tensor_tensor(o_all[:], o1[:], xtr_all[:],
                                            A.add)

            nc.sync.dma_start(out[:].rearrange("b d n -> d b n"),
                              o_all[:].rearrange("d (b n) -> d b n", b=B))

    nc.compile()
    return nc


def _get_program():
    key = (REPEAT, RELU_POLICY, ABLATE)
    if key not in _CACHE:
        _CACHE[key] = _build_program()
    return _CACHE[key]


_RUNNERS = {}


def _get_runner(nc):
    """Cached PJRT runner for a compiled program.

    ``bass_utils.run_bass_kernel_spmd`` builds a fresh ``shard_map`` +
    ``jax.jit`` closure on every call, so every kernel() call pays a full
    retrace/lower (~100-300ms, noisy).  Building the jitted executable once
    per program makes per-call wall time small and stable, which the
    repeat-delta timing method depends on.
    """
    if id(nc) in _RUNNERS:
        return _RUNNERS[id(nc)]

    import jax
    import concourse.mybir as mybir
    from concourse import bass2jax
    from jax.sharding import Mesh, PartitionSpec
    from jax.experimental.shard_map import shard_map

    bass2jax.install_neuronx_cc_hook()

    partition_name = (nc.partition_id_tensor.name
                      if nc.partition_id_tensor else None)
    in_names, out_names, out_avals, zero_shapes = [], [], [], []
    for alloc in nc.m.functions[0].allocations:
        if not isinstance(alloc, mybir.MemoryLocationSet):
            continue
        name = alloc.memorylocations[0].name
        if alloc.kind == "ExternalInput":
            if name != partition_name:
                in_names.append(name)
        elif alloc.kind == "ExternalOutput":
            shape = tuple(alloc.tensor_shape)
            dtype = mybir.dt.np(alloc.dtype)
            out_names.append(name)
            out_avals.append(jax.core.ShapedArray(shape, dtype))
            zero_shapes.append((shape, dtype))
    n_params = len(in_names)
    n_outs = len(out_avals)
    all_names = list(in_names) + list(out_names)
    if partition_name is not None:
        all_names.append(partition_name)
    donate = tuple(range(n_params, n_params + n_outs))

    def _body(*args):
        operands = list(args)
        if partition_name is not None:
            operands.append(bass2jax.partition_id_tensor())
        outs = bass2jax._bass_exec_p.bind(
            *operands,
            out_avals=tuple(out_avals),
            in_names=tuple(all_names),
            out_names=tuple(out_names),
            lowering_input_output_aliases=(),
            sim_require_finite=True,
            sim_require_nnan=True,
            nc=nc,
        )
        return tuple(outs)

    devices = jax.devices()[:NCORES]
    mesh = Mesh(np.asarray(devices), ("core",))
    in_specs = (PartitionSpec("core"),) * (n_params + n_outs)
    out_specs = (PartitionSpec("core"),) * n_outs
    sharded = jax.jit(
        shard_map(_body, mesh=mesh, in_specs=in_specs, out_specs=out_specs,
                  check_rep=False),
        donate_argnums=donate, keep_unused=True,
    )

    def run(in_maps):
        concat_in = [
            np.concatenate([np.asarray(m[name]) for m in in_maps], axis=0)
            for name in in_names
        ]
        concat_zeros = [
            np.zeros((NCORES * s[0], *s[1:]), dt) for s, dt in zero_shapes
        ]
        out_arrs = sharded(*concat_in, *concat_zeros)
        return [
            {
                name: np.asarray(out_arrs[i]).reshape(
                    NCORES, *zero_shapes[i][0])[c]
                for i, name in enumerate(out_names)
            }
            for c in range(NCORES)
        ]

    _RUNNERS[id(nc)] = run
    return run


def kernel(node_features, node_phases, adjacency,
           W1r, W1s, b1, W2, b2, Wg, bg, Wu1x, Wu1a, bu1, Wu2, bu2,
           _trace=False, _trace_kwargs=None):
    from concourse import bass_utils

    f4 = np.float32
    f2 = np.float16
    x = np.asarray(node_features, f4)
    ph = np.asarray(node_phases, f4)
    adj = np.asarray(adjacency)

    mask = (adj != 0)
    counts = np.maximum(mask.sum(axis=1), 1).astype(f4)           # (N,)
    cinv_full = (1.0 / counts)                                     # (N,)

    xt_full = np.ascontiguousarray(x.transpose(0, 2, 1))           # (B, D, N)
    p8_full = np.ascontiguousarray(
        np.concatenate([np.cos(ph), np.sin(ph)], axis=2).transpose(0, 2, 1)
    )                                                              # (B, 8, N)
    wg8 = np.concatenate([np.asarray(Wg, f4), np.asarray(Wg, f4)], axis=0)

    # Wg stacked per 32-row group + ones row (32g+8) for the mask passthrough
    wg8sh = np.zeros((D, D), f2)
    for g in range(4):
        wg8sh[32 * g:32 * g + 2 * O, :] = wg8.astype(f2)
        wg8sh[32 * g + 2 * O, :] = 1.0

    common = dict(xt=xt_full, wg8sh=wg8sh,
                  w2h=np.asarray(W2, f2))

    cidx = np.arange(NCH)
    in_maps = []
    for core in range(NCORES):
        lo, hi = core * NPC, (core + 1) * NPC
        m = dict(common)
        m["xtr"] = np.ascontiguousarray(
            np.concatenate([xt_full[bb][:, lo:hi] for bb in range(B)],
                           axis=1))
        # gate rhs: p8 rows (replicated per chunk) + per-receiver mask row
        mneg = (MASK_NEG * (~mask[lo:hi])).astype(f2)              # (NPC, N)
        p8m = np.zeros((B, D, NCH * N), f2)
        p8rep = np.tile(p8_full.astype(f2), (1, 1, NCH))           # (B,8,NCH*N)
        for g in range(4):
            p8m[:, 32 * g:32 * g + 2 * O, :] = p8rep
            p8m[:, 32 * g + 2 * O, :] = np.tile(
                mneg[cidx * 4 + g], (B, 1, 1)).reshape(B, NCH * N)
        m["p8m"] = np.ascontiguousarray(p8m)
        # receiver phases packed [32g+o, c] = P8r[o, 4c+g], + ones row
        p8r_core = p8_full[:, :, lo:hi].astype(f2)                 # (B, 8, NPC)
        p8rch = np.zeros((B, D, NCH), f2)
        for g in range(4):
            p8rch[:, 32 * g:32 * g + 2 * O, :] = p8r_core[:, :, cidx * 4 + g]
            p8rch[:, 32 * g + 2 * O, :] = 1.0
        m["p8rch"] = np.ascontiguousarray(p8rch)
        cinvb = np.broadcast_to(cinv_full[lo:hi][None, :], (D, NPC))
        m["blob"] = np.ascontiguousarray(np.concatenate(
            [np.asarray(W1r, f4), np.asarray(W1s, f4),
             np.asarray(Wu1x, f4), np.asarray(Wu1a, f4), np.asarray(Wu2, f4),
             cinvb, cinvb,
             np.asarray(b1, f4).reshape(D, 1), np.asarray(bg, f4).reshape(D, 1),
             np.asarray(b2, f4).reshape(D, 1),
             np.asarray(bu1, f4).reshape(D, 1),
             np.asarray(bu2, f4).reshape(D, 1)], axis=1))
        in_maps.append(m)

    nc = _get_program()
    if _trace:
        res = bass_utils.run_bass_kernel_spmd(
            nc, in_maps, list(range(NCORES)),
            trace=_trace, **(_trace_kwargs or {}))
        results = res.results
        kernel.last_results = res
    else:
        results = _get_runner(nc)(in_maps)

    out = np.empty((B, N, D), f4)
    for core in range(NCORES):
        lo, hi = core * NPC, (core + 1) * NPC
        out[:, lo:hi, :] = results[core]["out"].transpose(0, 2, 1)

    return out


# revision 7
# speedup vs baseline: 2.8113x; 2.8113x over previous
"""Trainium2 Bass kernel for MessagePassingWithPhase (v3).

Reference computation (B=2, N=512, D=128, O=4):
    recv = X @ W1r ; send = X @ W1s
    hidden[b,i,j,:]  = relu(recv[b,i] + send[b,j] + b1)
    messages         = hidden @ W2 + b2
    gate             = sigmoid(cos(phi_i - phi_j) @ Wg + bg)
    agg[b,i]         = sum_j mask[i,j] * (messages * gate)[b,i,j] / cnt_i
    out              = X + (relu(X@Wu1x + agg@Wu1a + bu1) @ Wu2 + bu2)

Mapping: 8 cores, receiver axis sharded (64 receivers/core, both batches),
feature-major (D=128 partitions, node index on the free axis).

v3 design (vs the v2 baseline's serialized PE->ACT->GPSIMD->DVE chain):
  * H = relu(send_j + recv_i + b1) is built directly from SBUF by the
    ACT/DVE/GPSIMD engines using the per-partition bias/scalar operand
    (bias = recv_i + b1 column) -- the v2 identity-matmul H-build (40% of
    all PE columns) is gone.
  * The gate linear, the adjacency mask (-48 saturation), and nothing else
    run as ONE fp16 K=32 matmul per receiver, row-tiled via tile_position
    so 4 receivers' gate matmuls execute concurrently in disjoint 32-row
    strips of the PE array.  The mask row rides along as contraction row
    32g+8 (stationary=1, rhs=-48*(1-mask)).
  * messages-evac + b2 + gate-multiply + the whole sum_j reduction collapse
    into ONE DVE scalar_tensor_tensor per receiver:
        out = (mq_psum + b2) * Gsb ; accum_out = sum_j -> araw column.
  * PSUM is double-buffered ((D,2N) gq/mq tags x2 bufs = 8 banks) so PE
    never waits on evacuation; relu H-builds are distributed across
    ACT/DVE/GPSIMD by RELU_POLICY to balance engine load.
"""
import os
import sys
import numpy as np

for _p in ("/opt/trn_rl_repo", "/root/.axon_site/_ro/trn_rl_repo"):
    if os.path.isdir(_p) and _p not in sys.path:
        sys.path.append(_p)

B, N, D, O = 2, 512, 128, 4
NCORES = 8
NPC = N // NCORES       # receivers per core (per batch)
NCH = NPC // 4          # 4-receiver groups per batch (p8m/wi chunking)
NC2 = NPC // 2          # 2-receiver PSUM chunks per batch
MASK_NEG = -48.0        # sigmoid(-48+eps) ~ 1e-20 -> masked-mean == sum

REPEAT = 1              # timing aid: loop trip count of the device For_i

# per-receiver engine for the relu H-build: s=ACT, v=DVE, g=GPSIMD.
# All H-builds run as a dependency-free phase up front (every engine's
# FIFO can drain them without cross-engine stalls), so the policy only
# balances total load.
RELU_POLICY = "sv"

# timing-ablation: stages to OMIT (timing experiments only; output wrong)
ABLATE = frozenset()

_CACHE = {}


def _build_program():
    import concourse.bacc as bacc
    import concourse.mybir as mybir
    import concourse.tile as tile

    f32 = mybir.dt.float32
    f16 = mybir.dt.float16
    A = mybir.AluOpType
    AF = mybir.ActivationFunctionType

    nc = bacc.Bacc("TRN2", debug=False)

    def din(name, shape, dt=f32):
        return nc.declare_dram_parameter(name, list(shape), dt, isOutput=False)

    xt = din("xt", (B, D, N))            # node features, transposed
    xtr = din("xtr", (D, B * NPC))       # receiver cols of xt, both batches
    p8m = din("p8m", (B, D, NCH * N), f16)   # gate rhs: P8 rows + mask row
    p8rch = din("p8rch", (B, D, NCH), f16)   # receiver phases packed + ones
    wg8sh = din("wg8sh", (D, D), f16)        # Wg stacked per group + ones
    w2h = din("w2h", (D, D), f16)
    # [w1r|w1s|wu1x|wu1a|wu2|cinv(B*NPC)|b1|bg|b2|bu1|bu2]
    NBLOB = 5 * D + B * NPC + 5
    blob = din("blob", (D, NBLOB))
    out = nc.declare_dram_parameter("out", [B, D, NPC], f32, isOutput=True)

    ab = ABLATE

    with tile.TileContext(nc) as tc:
        with (
            tc.tile_pool(name="const", bufs=1) as cp,
            tc.tile_pool(name="work", bufs=1) as wp,
            tc.tile_pool(name="psA", bufs=2, space="PSUM") as psA,
        ):
            def ct(dram, shape, dt=f32, tag=None):
                t = cp.tile(list(shape), dt, tag=tag, name=tag)
                nc.sync.dma_start(t[:], dram[:])
                return t

            blob_t = ct(blob, (D, NBLOB), tag="blob")
            w1r_t = blob_t[:, 0 * D:1 * D]
            w1s_t = blob_t[:, 1 * D:2 * D]
            wu1x_t = blob_t[:, 2 * D:3 * D]
            wu1a_t = blob_t[:, 3 * D:4 * D]
            wu2_t = blob_t[:, 4 * D:5 * D]
            cinv_t = blob_t[:, 5 * D:5 * D + B * NPC]
            bofs = 5 * D + B * NPC
            b1c = blob_t[:, bofs + 0:bofs + 1]
            bgc = blob_t[:, bofs + 1:bofs + 2]
            b2c = blob_t[:, bofs + 2:bofs + 3]
            bu1c = blob_t[:, bofs + 3:bofs + 4]
            bu2c = blob_t[:, bofs + 4:bofs + 5]

            w2h_t = ct(w2h, (D, D), f16, tag="w2h")
            wg8s_t = ct(wg8sh, (D, D), f16, tag="wg8sh")
            xt_t, p8m_t, p8rc_t = [], [], []
            for b in range(B):
                xt_t.append(ct(xt[b], (D, N), tag=f"xt{b}"))
                p8m_t.append(ct(p8m[b], (D, NCH * N), f16, tag=f"p8m{b}"))
                p8rc_t.append(ct(p8rch[b], (D, NCH), f16, tag=f"p8rc{b}"))
            xtr_all = ct(xtr, (D, B * NPC), tag="xtr")

            sendT = [cp.tile([D, N], f32, tag=f"send{b}", name=f"send{b}")
                     for b in range(B)]
            recvb1 = cp.tile([D, B * NPC], f32, tag="recvb1", name="recvb1")
            wi_pack = [cp.tile([D, NCH * D], f16, tag=f"wi{b}", name=f"wi{b}")
                       for b in range(B)]
            araw = cp.tile([D, B * NPC], f32, tag="araw", name="araw")
            o_all = cp.tile([D, B * NPC], f32, tag="o_all", name="o_all")

            relu_eng = {"s": nc.scalar, "v": nc.vector, "g": nc.gpsimd}

            chunks = [(b, c2) for b in range(B) for c2 in range(NC2)]
            NCHK = len(chunks)

            with tc.For_i(0, REPEAT, 1):
                # ---- projections ----
                for b in range(B if "proj" not in ab else 0):
                    s_ps = psA.tile([D, N], f32, tag="mq", name="s_ps")
                    nc.tensor.matmul(s_ps[:], w1s_t, xt_t[b][:],
                                     start=True, stop=True)
                    nc.scalar.copy(sendT[b][:], s_ps[:])
                if "proj" not in ab:
                    r_ps = psA.tile([D, B * NPC], f32, tag="gq", name="r_ps")
                    nc.tensor.matmul(r_ps[:], w1r_t, xtr_all[:],
                                     start=True, stop=True)
                    nc.scalar.add(recvb1[:], r_ps[:], b1c)
                else:
                    nc.vector.tensor_copy(recvb1[:], xtr_all[:])
                    for b in range(B):
                        nc.scalar.copy(sendT[b][:], xt_t[b][:])

                # ---- per-receiver gate stationaries (+ ones mask row) ----
                for b in range(B if "wi" not in ab else 0):
                    nc.vector.tensor_tensor(
                        wi_pack[b][:].rearrange("p (c d) -> p c d", c=NCH),
                        wg8s_t.unsqueeze(1).broadcast_to((D, NCH, D)),
                        p8rc_t[b][:].unsqueeze(2).broadcast_to((D, NCH, D)),
                        A.mult)
                if "wi" in ab:
                    for b in range(B):
                        nc.vector.tensor_copy(
                            wi_pack[b][:, 0:D],
                            wg8s_t)

                # ---- relu H-build phase: all 128 receivers, no deps ----
                Ht = {}
                for ci in range(NCHK):
                    bb, cc2 = chunks[ci]
                    Ht[ci] = wp.tile([D, 2 * N], f16, tag="H", bufs=NCHK,
                                     name="H")
                    if "relu" in ab:
                        nc.vector.memset(Ht[ci][:], 0.0)
                        continue
                    for k in range(2):
                        r = 2 * cc2 + k
                        col = bb * NPC + r
                        eng = relu_eng[RELU_POLICY[(ci * 2 + k)
                                                   % len(RELU_POLICY)]]
                        dst = Ht[ci][:, k * N:(k + 1) * N]
                        if eng is nc.scalar:
                            nc.scalar.activation(
                                dst, sendT[bb][:], AF.Relu,
                                bias=recvb1[:, col:col + 1])
                        else:
                            eng.tensor_scalar(
                                dst, sendT[bb][:],
                                recvb1[:, col:col + 1], 0.0,
                                A.add, A.max)

                # ---- chunk pipeline ----
                for ci in range(NCHK):
                    b, c2 = chunks[ci]

                    gq = psA.tile([D, 2 * N], f32, tag="gq", name="gq")
                    for k in range(2 if "gate" not in ab else 1):
                        r = 2 * c2 + k
                        g, c16 = r % 4, r // 4
                        nc.tensor.matmul(
                            gq[:, k * N:(k + 1) * N],
                            wi_pack[b][32 * g:32 * g + 32,
                                       c16 * D:(c16 + 1) * D],
                            p8m_t[b][32 * g:32 * g + 32,
                                     c16 * N:(c16 + 1) * N],
                            start=True, stop=True,
                            tile_position=(32 * g, 0))
                    if "gate" in ab:
                        nc.tensor.matmul(
                            gq[:, N:2 * N],
                            wi_pack[b][0:32, 0:D],
                            p8m_t[b][0:32, 0:N],
                            start=True, stop=True, tile_position=(0, 0))

                    Gsb = wp.tile([D, 2 * N], f16, tag="Gsb", bufs=2,
                                  name="Gsb")
                    if "sig" not in ab:
                        nc.scalar.activation(Gsb[:], gq[:], AF.Sigmoid,
                                             bias=bgc)
                    else:
                        nc.vector.memset(Gsb[:], 1.0)

                    mq = psA.tile([D, 2 * N], f32, tag="mq", name="mq")
                    for k in range(2 if "msg" not in ab else 1):
                        nc.tensor.matmul(
                            mq[:, k * N:(k + 1) * N], w2h_t[:],
                            Ht[ci][:, k * N:(k + 1) * N],
                            start=True, stop=True)
                    if "msg" in ab:
                        nc.tensor.matmul(
                            mq[:, N:2 * N], w2h_t[:], Ht[ci][:, 0:N],
                            start=True, stop=True)

                    scr = wp.tile([D, 2 * N], f16, tag="scr", bufs=2,
                                  name="scr")
                    for k in range(2 if "stt" not in ab else 0):
                        r = 2 * c2 + k
                        col = b * NPC + r
                        nc.vector.scalar_tensor_tensor(
                            scr[:, k * N:(k + 1) * N],
                            mq[:, k * N:(k + 1) * N],
                            b2c,
                            Gsb[:, k * N:(k + 1) * N],
                            A.add, A.mult,
                            accum_out=araw[:, col:col + 1])
                    del Ht[ci]
                if "stt" in ab:
                    nc.vector.memset(araw[:], 1.0)

                # ---- masked-mean scale + update MLP + residual ----
                if "tail" in ab:
                    nc.vector.tensor_copy(o_all[:], xtr_all[:])
                else:
                    aggt = wp.tile([D, B * NPC], f32, tag="aggt", name="aggt")
                    nc.vector.tensor_tensor(aggt[:], araw[:], cinv_t, A.mult)
                    u_ps = psA.tile([D, B * NPC], f32, tag="mq", name="u_ps")
                    nc.tensor.matmul(u_ps[:], wu1x_t, xtr_all[:],
                                     start=True, stop=False)
                    nc.tensor.matmul(u_ps[:], wu1a_t, aggt[:],
                                     start=False, stop=True)
                    hT = wp.tile([D, B * NPC], f32, tag="hT", name="hT")
                    nc.scalar.activation(hT[:], u_ps[:], AF.Relu, bias=bu1c)
                    o_ps = psA.tile([D, B * NPC], f32, tag="gq", name="o_ps")
                    nc.tensor.matmul(o_ps[:], wu2_t, hT[:],
                                     start=True, stop=True)
                    o1 = wp.tile([D, B * NPC], f32, tag="o1", name="o1")
                    nc.scalar.add(o1[:], o_ps[:], bu2c)
                    nc.vector.tensor_tensor(o_all[:], o1[:], xtr_all[:],
                                            A.add)

            nc.sync.dma_start(out[:].rearrange("b d n -> d b n"),
                              o_all[:].rearrange("d (b n) -> d b n", b=B))

    nc.compile()
    return nc


def _get_program():
    key = (REPEAT, RELU_POLICY, ABLATE)
    if key not in _CACHE:
        _CACHE[key] = _build_program()
    return _CACHE[key]


_RUNNERS = {}


def _get_runner(nc):
    """Cached PJRT runner for a compiled program.

    ``bass_utils.run_bass_kernel_spmd`` builds a fresh ``shard_map`` +
    ``jax.jit`` closure on every call, so every kernel() call pays a full
    retrace/lower (~100-300ms, noisy).  Building the jitted executable once
    per program makes per-call wall time small and stable, which the
    repeat-delta timing method depends on.
    """
    if id(nc) in _RUNNERS:
        return _RUNNERS[id(nc)]

    import jax
    import concourse.mybir as mybir
    from concourse import bass2jax
    from jax.sharding import Mesh, PartitionSpec
    from jax.experimental.shard_map import shard_map

    bass2jax.install_neuronx_cc_hook()

    partition_name = (nc.partition_id_tensor.name
                      if nc.partition_id_tensor else None)
    in_names, out_names, out_avals, zero_shapes = [], [], [], []
    for alloc in nc.m.functions[0].allocations:
        if not isinstance(alloc, mybir.MemoryLocationSet):
            continue
        name = alloc.memorylocations[0].name
        if alloc.kind == "ExternalInput":
            if name != partition_name:
                in_names.append(name)
        elif alloc.kind == "ExternalOutput":
            shape = tuple(alloc.tensor_shape)
            dtype = mybir.dt.np(alloc.dtype)
            out_names.append(name)
            out_avals.append(jax.core.ShapedArray(shape, dtype))
            zero_shapes.append((shape, dtype))
    n_params = len(in_names)
    n_outs = len(out_avals)
    all_names = list(in_names) + list(out_names)
    if partition_name is not None:
        all_names.append(partition_name)
    donate = tuple(range(n_params, n_params + n_outs))

    def _body(*args):
        operands = list(args)
        if partition_name is not None:
            operands.append(bass2jax.partition_id_tensor())
        outs = bass2jax._bass_exec_p.bind(
            *operands,
            out_avals=tuple(out_avals),
            in_names=tuple(all_names),
            out_names=tuple(out_names),
            lowering_input_output_aliases=(),
            sim_require_finite=True,
            sim_require_nnan=True,
            nc=nc,
        )
        return tuple(outs)

    devices = jax.devices()[:NCORES]
    mesh = Mesh(np.asarray(devices), ("core",))
    in_specs = (PartitionSpec("core"),) * (n_params + n_outs)
    out_specs = (PartitionSpec("core"),) * n_outs
    sharded = jax.jit(
        shard_map(_body, mesh=mesh, in_specs=in_specs, out_specs=out_specs,
                  check_rep=False),
        donate_argnums=donate, keep_unused=True,
    )

    def run(in_maps):
        concat_in = [
            np.concatenate([np.asarray(m[name]) for m in in_maps], axis=0)
            for name in in_names
        ]
        concat_zeros = [
            np.zeros((NCORES * s[0], *s[1:]), dt) for s, dt in zero_shapes
        ]
        out_arrs = sharded(*concat_in, *concat_zeros)
        return [
            {
                name: np.asarray(out_arrs[i]).reshape(
                    NCORES, *zero_shapes[i][0])[c]
                for i, name in enumerate(out_names)
            }
            for c in range(NCORES)
        ]

    _RUNNERS[id(nc)] = run
    return run


def kernel(node_features, node_phases, adjacency,
           W1r, W1s, b1, W2, b2, Wg, bg, Wu1x, Wu1a, bu1, Wu2, bu2,
           _trace=False, _trace_kwargs=None):
    from concourse import bass_utils

    f4 = np.float32
    f2 = np.float16
    x = np.asarray(node_features, f4)
    ph = np.asarray(node_phases, f4)
    adj = np.asarray(adjacency)

    mask = (adj != 0)
    counts = np.maximum(mask.sum(axis=1), 1).astype(f4)           # (N,)
    cinv_full = (1.0 / counts)                                     # (N,)

    xt_full = np.ascontiguousarray(x.transpose(0, 2, 1))           # (B, D, N)
    p8_full = np.ascontiguousarray(
        np.concatenate([np.cos(ph), np.sin(ph)], axis=2).transpose(0, 2, 1)
    )                                                              # (B, 8, N)
    wg8 = np.concatenate([np.asarray(Wg, f4), np.asarray(Wg, f4)], axis=0)

    # Wg stacked per 32-row group + ones row (32g+8) for the mask passthrough
    wg8sh = np.zeros((D, D), f2)
    for g in range(4):
        wg8sh[32 * g:32 * g + 2 * O, :] = wg8.astype(f2)
        wg8sh[32 * g + 2 * O, :] = 1.0

    common = dict(xt=xt_full, wg8sh=wg8sh,
                  w2h=np.asarray(W2, f2))

    cidx = np.arange(NCH)
    in_maps = []
    for core in range(NCORES):
        lo, hi = core * NPC, (core + 1) * NPC
        m = dict(common)
        m["xtr"] = np.ascontiguousarray(
            np.concatenate([xt_full[bb][:, lo:hi] for bb in range(B)],
                           axis=1))
        # gate rhs: p8 rows (replicated per chunk) + per-receiver mask row
        mneg = (MASK_NEG * (~mask[lo:hi])).astype(f2)              # (NPC, N)
        p8m = np.zeros((B, D, NCH * N), f2)
        p8rep = np.tile(p8_full.astype(f2), (1, 1, NCH))           # (B,8,NCH*N)
        for g in range(4):
            p8m[:, 32 * g:32 * g + 2 * O, :] = p8rep
            p8m[:, 32 * g + 2 * O, :] = np.tile(
                mneg[cidx * 4 + g], (B, 1, 1)).reshape(B, NCH * N)
        m["p8m"] = np.ascontiguousarray(p8m)
        # receiver phases packed [32g+o, c] = P8r[o, 4c+g], + ones row
        p8r_core = p8_full[:, :, lo:hi].astype(f2)                 # (B, 8, NPC)
        p8rch = np.zeros((B, D, NCH), f2)
        for g in range(4):
            p8rch[:, 32 * g:32 * g + 2 * O, :] = p8r_core[:, :, cidx * 4 + g]
            p8rch[:, 32 * g + 2 * O, :] = 1.0
        m["p8rch"] = np.ascontiguousarray(p8rch)
        cinvb = np.broadcast_to(cinv_full[lo:hi][None, :], (D, NPC))
        m["blob"] = np.ascontiguousarray(np.concatenate(
            [np.asarray(W1r, f4), np.asarray(W1s, f4),
             np.asarray(Wu1x, f4), np.asarray(Wu1a, f4), np.asarray(Wu2, f4),
             cinvb, cinvb,
             np.asarray(b1, f4).reshape(D, 1), np.asarray(bg, f4).reshape(D, 1),
             np.asarray(b2, f4).reshape(D, 1),
             np.asarray(bu1, f4).reshape(D, 1),
             np.asarray(bu2, f4).reshape(D, 1)], axis=1))
        in_maps.append(m)

    nc = _get_program()
    if _trace:
        res = bass_utils.run_bass_kernel_spmd(
            nc, in_maps, list(range(NCORES)),
            trace=_trace, **(_trace_kwargs or {}))
        results = res.results
        kernel.last_results = res
    else:
        results = _get_runner(nc)(in_maps)

    out = np.empty((B, N, D), f4)
    for core in range(NCORES):
        lo, hi = core * NPC, (core + 1) * NPC
        out[:, lo:hi, :] = results[core]["out"].transpose(0, 2, 1)

    return out
